# revision 8
# baseline (speedup 1.0000x reference)
"""DGCNN+GAT kernel for 8x Trainium2 NeuronCores (Bass/Tile).

Math (from the reference nn.Module):
  - The EdgeConv MLP output is discarded by the reference; only the kNN graph,
    the GAT layer, elu, per-sample mean and the final fc affect the output.
  - Global kNN over N=256*64=16384 nodes, k=4 (plus self loop in GAT).
  - Ranking trick: for row i, rank candidates j by s[i,j] = x_i.x_j - |x_j|^2/2
    (equals (|x_i|^2 - d_ij)/2, so ordering == nearest-neighbor ordering and
    s[i,i] is the row max -> top-5 of s = {self, 4 nearest}).
  - s is computed on the PE as one fused matmul with contraction 33:
    lhsT = [x_i; -1], rhs = [x_j; |x_j|^2/2].
  - Per 128-row tile, DVE max8 gives the top-8 values, max_index their global
    indices; slots 0..4 = {self, nn1..nn4}.
  - A 192-float row table [h(128) | asrc(4) | pad] in DRAM is gathered with
    gpsimd dma_gather (640 rows per tile) to feed the GAT attention.

Sharding: data-parallel over rows; core c owns nodes [c*2048, (c+1)*2048) =
32 batch samples. Weights and x are replicated; output [32, 2] per core is
concatenated on the host.
"""

import os
import numpy as np

import concourse.bass as bass
import concourse.bacc as bacc
import concourse.mybir as mybir
from concourse.tile import TileContext
from concourse.bass_utils import run_bass_kernel_spmd

F32 = mybir.dt.float32
I16 = mybir.dt.int16
U16 = mybir.dt.uint16
ALU = mybir.AluOpType
ACT = mybir.ActivationFunctionType
AX = mybir.AxisListType

N_CORES = 8
B, E, C = 256, 64, 32
N = B * E                     # 16384 nodes
HEADS, HID = 4, 32
F = HEADS * HID               # 128
LOCAL = N // N_CORES          # 2048 rows per core
TILES = LOCAL // 128          # 16 row tiles per core
ROW = 192                     # htab row in f32 (768B, /256B for dma_gather)
KSLOT = 5                     # self + 4 neighbors
NEG_SLOPE = 0.2


def build_bass():
    nc = bacc.Bacc(None, target_bir_lowering=False)

    # ---- I/O ----
    xf_d = nc.dram_tensor("xf", [N, C], F32, kind="ExternalInput")         # node major
    xT_d = nc.dram_tensor("xT", [C, N], F32, kind="ExternalInput")         # feature major
    xTloc_d = nc.dram_tensor("xTloc", [C, LOCAL], F32, kind="ExternalInput")
    gatw_d = nc.dram_tensor("gat_w", [C, F], F32, kind="ExternalInput")
    asrc_d = nc.dram_tensor("gat_asrc", [1, F], F32, kind="ExternalInput")
    adst_d = nc.dram_tensor("gat_adst", [1, F], F32, kind="ExternalInput")
    gatb_d = nc.dram_tensor("gat_b", [1, F], F32, kind="ExternalInput")
    fcw_d = nc.dram_tensor("fc_w", [F, 2], F32, kind="ExternalInput")
    fcb_d = nc.dram_tensor("fc_b", [1, 2], F32, kind="ExternalInput")
    y_d = nc.dram_tensor("y", [B // N_CORES, 2], F32, kind="ExternalOutput")

    with TileContext(nc) as tc:
        with (
            tc.tile_pool(name="const", bufs=1) as const,
            tc.tile_pool(name="setup", bufs=1) as setup,
            tc.tile_pool(name="spool", bufs=2) as spool,
            tc.tile_pool(name="apool", bufs=3) as apool,
            tc.tile_pool(name="hlhs", bufs=3) as hlhs,
            tc.tile_pool(name="hst", bufs=3) as hstp,
            tc.tile_pool(name="k8", bufs=3) as k8p,
            tc.tile_pool(name="idx", bufs=2) as idxp,
            tc.tile_pool(name="gpool", bufs=2) as gpool,
            tc.tile_pool(name="att", bufs=1) as attp,
            tc.tile_pool(name="ps_s", bufs=2, space="PSUM") as ps_s,
            tc.tile_pool(name="ps_h", bufs=2, space="PSUM") as ps_h,
            tc.tile_pool(name="ps_o", bufs=1, space="PSUM") as ps_o,
            tc.tile_pool(name="dram", bufs=1, space="DRAM") as dpool,
            tc.tile_pool(name="dscr", bufs=2, space="DRAM") as dscr,
        ):
            htab = dpool.tile([N, ROW], F32)            # gather table
            a_dram = dpool.tile([C + 1, N], F32)        # [x_j ; |x_j|^2/2]

            # ================= setup =================
            # b_t = [x_loc ; -1] (contraction-major lhsT source for kNN matmul)
            b_t = const.tile([C + 1, LOCAL], F32)
            nc.sync.dma_start(out=b_t[0:C, :], in_=xTloc_d[:, :])
            nc.vector.memset(b_t[C : C + 1, :], -1.0)

            # squared norms / 2 -> a_dram row 32; rows 0:32 <- xT
            nc.sync.dma_start(out=a_dram[0:C, :], in_=xT_d[:, :])
            xnm = setup.tile([128, 128 * C], F32)
            nc.sync.dma_start(
                out=xnm[:, :], in_=xf_d.rearrange("(p g) c -> p (g c)", p=128)
            )
            nc.scalar.activation(xnm[:, :], xnm[:, :], ACT.Square)
            sqnm = setup.tile([128, 128], F32)
            nc.vector.tensor_reduce(
                sqnm[:, :],
                xnm.rearrange("p (g c) -> p g c", g=128),
                axis=AX.X,
                op=ALU.add,
            )
            nc.scalar.mul(sqnm[:, :], sqnm[:, :], 0.5)
            nc.sync.dma_start(
                out=a_dram[C, :].rearrange("(p g) -> p g", p=128), in_=sqnm[:, :]
            )

            # wcombo = [gat_w | Wsrc | Wdst]  (Wsrc[c,h] = sum_d W[c,h*32+d]*a_src[h,d])
            wcombo = const.tile([C, F + 8], F32)
            nc.sync.dma_start(out=wcombo[:, 0:F], in_=gatw_d[:, :])
            avec = const.tile([1, 2 * F], F32)
            nc.sync.dma_start(out=avec[:, 0:F], in_=asrc_d[:, :])
            nc.sync.dma_start(out=avec[:, F : 2 * F], in_=adst_d[:, :])
            avec_bc = const.tile([C, 2 * F], F32)
            nc.gpsimd.partition_broadcast(avec_bc[:, :], avec[:, :], channels=C)
            wtmp = const.tile([C, F], F32)
            for which in range(2):  # 0 = src, 1 = dst
                nc.vector.tensor_mul(
                    wtmp[:, :], wcombo[:, 0:F], avec_bc[:, which * F : (which + 1) * F]
                )
                nc.vector.tensor_reduce(
                    wcombo[:, F + 4 * which : F + 4 * which + 4],
                    wtmp.rearrange("p (h d) -> p h d", h=HEADS),
                    axis=AX.X,
                    op=ALU.add,
                )

            # constants
            sel = const.tile([128, 2], F32)
            nc.vector.memset(sel[:, :], 0.0)
            nc.vector.memset(sel[0:64, 0:1], 1.0)
            nc.vector.memset(sel[64:128, 1:2], 1.0)
            gatb_bc = const.tile([128, F], F32)
            gb1 = const.tile([1, F], F32)
            nc.sync.dma_start(out=gb1[:, :], in_=gatb_d[:, :])
            nc.gpsimd.partition_broadcast(gatb_bc[:, :], gb1[:, :], channels=128)
            fcw_sb = const.tile([F, 2], F32)
            nc.sync.dma_start(out=fcw_sb[:, :], in_=fcw_d[:, :])
            fcb1 = const.tile([1, 2], F32)
            nc.sync.dma_start(out=fcb1[:, :], in_=fcb_d[:, :])
            fcb_bc = const.tile([B // N_CORES, 2], F32)
            nc.gpsimd.partition_broadcast(fcb_bc[:, :], fcb1[:, :], channels=B // N_CORES)
            adst_sb = const.tile([128, TILES * HEADS], F32)

            # persistent PSUM accum for per-sample sums (feature-major)
            ps_meanT = ps_o.tile([F, 2 * TILES], F32)

            # ================= phase H: h table =================
            # h|asrc|adst for all N nodes; rows -> htab[:, 0:132]
            NCHUNK = N // 128          # 128 chunks of 128 nodes
            GRP = 3
            for g0 in range(0, NCHUNK, GRP):
                ks = range(g0, min(g0 + GRP, NCHUNK))
                ps = ps_h.tile([128, 136 * GRP], F32, tag="hps")
                for j, k in enumerate(ks):
                    lh = hlhs.tile([C, 128], F32)
                    nc.sync.dma_start(out=lh[:, :], in_=xT_d[:, k * 128 : (k + 1) * 128])
                    nc.tensor.matmul(
                        ps[:, j * 136 : j * 136 + 136],
                        lhsT=lh[:, :],
                        rhs=wcombo[:, :],
                        start=True,
                        stop=True,
                    )
                hs = hstp.tile([128, ROW * GRP], F32)
                nc.gpsimd.memset(
                    hs.rearrange("p (k w) -> p k w", w=ROW)[:, 0 : len(ks), 132:ROW], 0.0
                )
                nc.scalar.copy(
                    hs.rearrange("p (k w) -> p k w", w=ROW)[:, 0 : len(ks), 0:132],
                    ps.rearrange("p (k w) -> p k w", w=136)[:, 0 : len(ks), 0:132],
                )
                for j, k in enumerate(ks):
                    nc.sync.dma_start(
                        out=htab[k * 128 : (k + 1) * 128, :],
                        in_=hs[:, j * ROW : (j + 1) * ROW],
                    )

            # local adst slices for the attention phase
            for t in range(TILES):
                ps2 = ps_h.tile([128, 136 * GRP], F32, tag="hps")
                nc.tensor.matmul(
                    ps2[:, 0:4],
                    lhsT=b_t[0:C, t * 128 : (t + 1) * 128],
                    rhs=wcombo[:, F + 4 : F + 8],
                    start=True,
                    stop=True,
                )
                nc.scalar.copy(adst_sb[:, t * 4 : t * 4 + 4], ps2[:, 0:4])

            # ================= kNN + attention per tile =================
            for t in range(TILES):
                s_sb = spool.tile([128, N], F32)
                for r in range(N // 1024):
                    ps = ps_s.tile([128, 1024], F32)
                    ach = apool.tile([C + 1, 1024], F32)
                    dma_eng = nc.sync if r % 2 == 0 else nc.scalar
                    dma_eng.dma_start(
                        out=ach[:, :], in_=a_dram[:, r * 1024 : (r + 1) * 1024]
                    )
                    for hh in range(2):
                        nc.tensor.matmul(
                            ps[:, hh * 512 : (hh + 1) * 512],
                            lhsT=b_t[:, t * 128 : (t + 1) * 128],
                            rhs=ach[:, hh * 512 : (hh + 1) * 512],
                            start=True,
                            stop=True,
                        )
                    nc.scalar.copy(s_sb[:, r * 1024 : (r + 1) * 1024], ps[:, :])

                v8 = k8p.tile([128, 8], F32)
                nc.vector.max(out=v8[:, :], in_=s_sb[:, :])
                i8 = k8p.tile([128, 8], U16)
                nc.vector.max_index(out=i8[:, :], in_max=v8[:, :], in_values=s_sb[:, :])

                # wrap indices for dma_gather: L[i= j*128+p] at (i%16, i//16),
                # replicated into all 8 16-partition groups, via a DRAM bounce.
                iscr = dscr.tile([128 * KSLOT], I16)
                nc.gpsimd.dma_start(
                    out=iscr.rearrange("(j p) -> p j", p=128),
                    in_=i8[:, 0:KSLOT].bitcast(I16),
                )
                lrep = idxp.tile([128, 8 * KSLOT], I16)
                for rep in range(8):
                    nc.gpsimd.dma_start(
                        out=lrep[16 * rep : 16 * (rep + 1), :],
                        in_=iscr.rearrange("(s q) -> q s", q=16),
                    )
                g_t = gpool.tile([128, KSLOT * ROW], F32)
                nc.gpsimd.dma_gather(
                    out_ap=g_t.rearrange("p (j e) -> p j e", j=KSLOT),
                    in_ap=htab[:, :],
                    idxs_ap=lrep[:, :],
                    num_idxs=128 * KSLOT,
                    num_idxs_reg=128 * KSLOT,
                    elem_size=ROW,
                )

                # ---- attention ----
                gv = g_t.rearrange("p (j e) -> p j e", j=KSLOT)
                e_t = attp.tile([128, KSLOT * HEADS], F32)
                nc.vector.tensor_add(
                    e_t.rearrange("p (j h) -> p j h", j=KSLOT),
                    gv[:, :, F : F + 4],
                    adst_sb[:, t * 4 : t * 4 + 4]
                    .unsqueeze(1)
                    .to_broadcast([128, KSLOT, HEADS]),
                )
                # leaky relu = max(0.2*e, e)
                lr = attp.tile([128, KSLOT * HEADS], F32)
                nc.vector.scalar_tensor_tensor(
                    lr[:, :], e_t[:, :], NEG_SLOPE, e_t[:, :], op0=ALU.mult, op1=ALU.max
                )
                pe = attp.tile([128, KSLOT * HEADS], F32)
                nc.scalar.activation(pe[:, :], lr[:, :], ACT.Exp)
                zz = attp.tile([128, HEADS], F32)
                nc.vector.tensor_reduce(
                    zz[:, :],
                    pe.rearrange("p (j h) -> p h j", j=KSLOT),
                    axis=AX.X,
                    op=ALU.add,
                )
                zr = attp.tile([128, HEADS], F32)
                nc.vector.reciprocal(zr[:, :], zz[:, :])
                w1 = attp.tile([128, KSLOT * F], F32)
                nc.vector.tensor_mul(
                    w1.rearrange("p (j h d) -> p j h d", j=KSLOT, h=HEADS),
                    g_t.rearrange("p (j hh d) -> p j hh d", j=KSLOT, d=HID)[:, :, 0:HEADS, :],
                    pe.rearrange("p (j h) -> p j h", j=KSLOT).to_broadcast(
                        [128, KSLOT, HEADS, HID]
                    ),
                )
                att_t = attp.tile([128, F], F32)
                nc.vector.tensor_reduce(
                    att_t.rearrange("p (h d) -> p h d", h=HEADS),
                    w1.rearrange("p (j h d) -> p h d j", j=KSLOT, h=HEADS),
                    axis=AX.X,
                    op=ALU.add,
                )
                z_t = attp.tile([128, F], F32)
                nc.vector.tensor_mul(
                    z_t.rearrange("p (h d) -> p h d", h=HEADS),
                    att_t.rearrange("p (h d) -> p h d", h=HEADS),
                    zr.to_broadcast([128, HEADS, HID]),
                )
                nc.vector.tensor_add(z_t[:, :], z_t[:, :], gatb_bc[:, :])
                # elu(z) = max(z,0) + exp(min(z,0)) - 1
                zn = attp.tile([128, F], F32)
                nc.vector.tensor_scalar_min(zn[:, :], z_t[:, :], 0.0)
                en = attp.tile([128, F], F32)
                nc.scalar.activation(en[:, :], zn[:, :], ACT.Exp)
                zp = attp.tile([128, F], F32)
                nc.vector.tensor_scalar_max(zp[:, :], z_t[:, :], 0.0)
                zelu = attp.tile([128, F], F32)
                nc.vector.scalar_tensor_tensor(
                    zelu[:, :], en[:, :], -1.0, zp[:, :], op0=ALU.add, op1=ALU.add
                )
                # per-sample sums (each tile = 2 samples), feature-major
                nc.tensor.matmul(
                    ps_meanT[:, 2 * t : 2 * t + 2],
                    lhsT=zelu[:, :],
                    rhs=sel[:, :],
                    start=True,
                    stop=True,
                )

            # ================= output =================
            meanT = const.tile([F, 2 * TILES], F32)
            nc.scalar.copy(meanT[:, :], ps_meanT[:, :])
            ps_fc = ps_o.tile([B // N_CORES, 2], F32)
            nc.tensor.matmul(
                ps_fc[:, :], lhsT=meanT[:, :], rhs=fcw_sb[:, :], start=True, stop=True
            )
            y_pre = const.tile([B // N_CORES, 2], F32)
            nc.scalar.mul(y_pre[:, :], ps_fc[:, :], 1.0 / E)
            y_sb = const.tile([B // N_CORES, 2], F32)
            nc.vector.tensor_add(y_sb[:, :], y_pre[:, :], fcb_bc[:, :])
            nc.sync.dma_start(out=y_d[:, :], in_=y_sb[:, :])

    nc.finalize()
    return nc


_NC = None


def _get_nc():
    global _NC
    if _NC is None:
        _NC = build_bass()
    return _NC


def _prep_inputs(inputs):
    x = np.ascontiguousarray(inputs["x"], dtype=np.float32)
    xf = np.ascontiguousarray(x.reshape(N, C))
    xT = np.ascontiguousarray(xf.T)
    shared = {
        "xf": xf,
        "xT": xT,
        "gat_w": np.ascontiguousarray(inputs["gat_w"], np.float32),
        "gat_asrc": np.ascontiguousarray(
            np.asarray(inputs["gat_asrc"], np.float32).reshape(1, F)
        ),
        "gat_adst": np.ascontiguousarray(
            np.asarray(inputs["gat_adst"], np.float32).reshape(1, F)
        ),
        "gat_b": np.ascontiguousarray(
            np.asarray(inputs["gat_b"], np.float32).reshape(1, F)
        ),
        "fc_w": np.ascontiguousarray(inputs["fc_w"], np.float32),
        "fc_b": np.ascontiguousarray(
            np.asarray(inputs["fc_b"], np.float32).reshape(1, 2)
        ),
    }
    maps = []
    for c in range(N_CORES):
        m = dict(shared)
        m["xTloc"] = np.ascontiguousarray(xT[:, c * LOCAL : (c + 1) * LOCAL])
        maps.append(m)
    return maps


def kernel(**inputs) -> np.ndarray:
    nc = _get_nc()
    maps = _prep_inputs(inputs)
    res = run_bass_kernel_spmd(
        nc,
        maps,
        core_ids=list(range(N_CORES)),
        trace=bool(int(os.environ.get("KERNEL_TRACE", "0"))),
    )
    y = np.concatenate([res.results[c]["y"] for c in range(N_CORES)], axis=0)
    kernel.last_exec_time_ns = res.exec_time_ns
    kernel.last_results = res
    return y


# revision 20
# speedup vs baseline: 1.1043x; 1.1043x over previous
"""DGCNN+GAT kernel for 8x Trainium2 NeuronCores (Bass/Tile).

Math (from the reference nn.Module):
  - The EdgeConv MLP output is discarded by the reference; only the kNN graph,
    the GAT layer, elu, per-sample mean and the final fc affect the output.
  - Global kNN over N=256*64=16384 nodes, k=4 (plus self loop in GAT).
  - Ranking trick: for row i, rank candidates j by s[i,j] = x_i.x_j - |x_j|^2/2
    (equal ordering to nearest-neighbor ordering; s[i,i] is the row max, so
    top-5 of s = {self, 4 nearest}).
  - Coarse pass: s is computed on the PE as one fused bf16 matmul with
    contraction 33 (lhsT = [x_i; -1], rhs = [x_j; |x_j|^2/2]); DVE max8 +
    find_index8 give the top-8 coarse candidates per row (bf16-accurate
    ranking; top-8 provably covers the exact top-5 with large margin).
  - Exact pass: the 8 candidates' x_j rows are gathered (dma_gather) and
    rescored exactly in fp32: s8 = sum_c (x_i - x_j/2)*x_j.  The top-5 of s8
    are selected by masking the softmax of the other slots to zero - no index
    compaction needed, attention runs over all 8 slots.
  - A 192-float row table [h(128) | asrc(4) | x(32) | pad] in DRAM feeds both
    gathers (elem_step=192 with elem_size=64 for the rescore slice).

Sharding: data-parallel over rows; core c owns nodes [c*2048, (c+1)*2048) =
32 batch samples. Weights and x are replicated; output [32, 2] per core is
concatenated on the host.
"""

import os
import numpy as np

import concourse.bass as bass
import concourse.bacc as bacc
import concourse.mybir as mybir
from concourse.tile import TileContext
from concourse.bass_utils import run_bass_kernel_spmd

F32 = mybir.dt.float32
BF16 = mybir.dt.bfloat16
I16 = mybir.dt.int16
U16 = mybir.dt.uint16
ALU = mybir.AluOpType
ACT = mybir.ActivationFunctionType
AX = mybir.AxisListType

N_CORES = 8
B, E, C = 256, 64, 32
N = B * E                     # 16384 nodes
HEADS, HID = 4, 32
F = HEADS * HID               # 128
LOCAL = N // N_CORES          # 2048 rows per core
TILES = LOCAL // 128          # 16 row tiles per core
ROW = 192                     # htab row in f32 (768B, /256B for dma_gather)
KS = 8                        # coarse slots (max8 width)
NEG_SLOPE = 0.2
BIG = 30000.0


def build_bass():
    nc = bacc.Bacc(None, target_bir_lowering=False)

    # ---- I/O ----
    xf_d = nc.dram_tensor("xf", [N, C], F32, kind="ExternalInput")         # node major
    xT_d = nc.dram_tensor("xT", [C, N], F32, kind="ExternalInput")         # feature major
    xTloc_d = nc.dram_tensor("xTloc", [C, LOCAL], F32, kind="ExternalInput")
    xfloc_d = nc.dram_tensor("xfloc", [LOCAL, C], F32, kind="ExternalInput")
    gatw_d = nc.dram_tensor("gat_w", [C, F], F32, kind="ExternalInput")
    asrc_d = nc.dram_tensor("gat_asrc", [1, F], F32, kind="ExternalInput")
    adst_d = nc.dram_tensor("gat_adst", [1, F], F32, kind="ExternalInput")
    gatb_d = nc.dram_tensor("gat_b", [1, F], F32, kind="ExternalInput")
    fcw_d = nc.dram_tensor("fc_w", [F, 2], F32, kind="ExternalInput")
    fcb_d = nc.dram_tensor("fc_b", [1, 2], F32, kind="ExternalInput")
    y_d = nc.dram_tensor("y", [B // N_CORES, 2], F32, kind="ExternalOutput")
    debug = bool(int(os.environ.get("KERNEL_DEBUG", "0")))
    if debug:
        dbg_i8 = nc.dram_tensor("dbg_i8", [LOCAL, 8], U16, kind="ExternalOutput")
        dbg_s8 = nc.dram_tensor("dbg_s8", [LOCAL, 8], F32, kind="ExternalOutput")
        dbg_m01 = nc.dram_tensor("dbg_m01", [LOCAL, 8], F32, kind="ExternalOutput")

    with TileContext(nc) as tc:
        with (
            tc.tile_pool(name="const", bufs=1) as const,
            tc.tile_pool(name="setup", bufs=1) as setup,
            tc.tile_pool(name="spool", bufs=2) as spool,
            tc.tile_pool(name="apool", bufs=3) as apool,
            tc.tile_pool(name="hlhs", bufs=3) as hlhs,
            tc.tile_pool(name="hst", bufs=2) as hstp,
            tc.tile_pool(name="k8", bufs=3) as k8p,
            tc.tile_pool(name="idx", bufs=2) as idxp,
            tc.tile_pool(name="gpool", bufs=2) as gpool,
            tc.tile_pool(name="att", bufs=1) as attp,
            tc.tile_pool(name="ps_s", bufs=2, space="PSUM") as ps_s,
            tc.tile_pool(name="ps_h", bufs=2, space="PSUM") as ps_h,
            tc.tile_pool(name="ps_o", bufs=1, space="PSUM") as ps_o,
            tc.tile_pool(name="dram", bufs=1, space="DRAM") as dpool,
            tc.tile_pool(name="dscr", bufs=2, space="DRAM") as dscr,
        ):
            htab = dpool.tile([N, ROW], F32)            # gather table
            a_dram = dpool.tile([C + 1, N], BF16)       # bf16 [x_j ; |x_j|^2/2]

            # ================= setup =================
            # xTloc fp32 resident (adst matmuls); b_t = bf16 [x_loc ; -1]
            xTloc_sb = const.tile([C, LOCAL], F32)
            nc.sync.dma_start(out=xTloc_sb[:, :], in_=xTloc_d[:, :])
            b_t = const.tile([C + 1, LOCAL], BF16)
            nc.vector.tensor_copy(b_t[0:C, :], xTloc_sb[:, :])
            nc.vector.memset(b_t[C : C + 1, :], -1.0)

            # half squared norms (fp32), transposed layout: sq2h[p, g] =
            # |x_{g*128+p}|^2/2 (chunk g's nodes live in column g).
            sq2h = const.tile([128, 128], F32)
            for half in range(2):
                xnm = setup.tile([128, 64 * C], F32, tag="stg32")
                nc.sync.dma_start(
                    out=xnm[:, :],
                    in_=xf_d.rearrange("(g p) c -> p g c", p=128)[
                        :, half * 64 : (half + 1) * 64, :
                    ],
                )
                nc.scalar.activation(xnm[:, :], xnm[:, :], ACT.Square)
                nc.vector.tensor_reduce(
                    sq2h[:, half * 64 : (half + 1) * 64],
                    xnm.rearrange("p (g c) -> p g c", c=C),
                    axis=AX.X,
                    op=ALU.add,
                )
            nc.scalar.mul(sq2h[:, :], sq2h[:, :], 0.5)
            sqbf = setup.tile([128, 128], BF16)
            nc.vector.tensor_copy(sqbf[:, :], sq2h[:, :])
            nc.sync.dma_start(
                out=a_dram[C, :].rearrange("(g p) -> p g", p=128), in_=sqbf[:, :]
            )
            # a_dram rows 0:32 = bf16(xT), via cast rounds
            for r in range(8):
                stg = setup.tile([C, 2048], F32, tag="stg32")
                nc.sync.dma_start(out=stg[:, :], in_=xT_d[:, r * 2048 : (r + 1) * 2048])
                stgb = setup.tile([C, 2048], BF16, tag="stgbf")
                nc.vector.tensor_copy(stgb[:, :], stg[:, :])
                nc.sync.dma_start(
                    out=a_dram[0:C, r * 2048 : (r + 1) * 2048], in_=stgb[:, :]
                )

            # wcombo = [gat_w | Wsrc | Wdst]  (Wsrc[c,h] = sum_d W[c,h*32+d]*a_src[h,d])
            wcombo = const.tile([C, F + 8], F32)
            nc.sync.dma_start(out=wcombo[:, 0:F], in_=gatw_d[:, :])
            avec = const.tile([1, 2 * F], F32)
            nc.sync.dma_start(out=avec[:, 0:F], in_=asrc_d[:, :])
            nc.sync.dma_start(out=avec[:, F : 2 * F], in_=adst_d[:, :])
            avec_bc = const.tile([C, 2 * F], F32)
            nc.gpsimd.partition_broadcast(avec_bc[:, :], avec[:, :], channels=C)
            wtmp = const.tile([C, F], F32)
            for which in range(2):  # 0 = src, 1 = dst
                nc.vector.tensor_mul(
                    wtmp[:, :], wcombo[:, 0:F], avec_bc[:, which * F : (which + 1) * F]
                )
                nc.vector.tensor_reduce(
                    wcombo[:, F + 4 * which : F + 4 * which + 4],
                    wtmp.rearrange("p (h d) -> p h d", h=HEADS),
                    axis=AX.X,
                    op=ALU.add,
                )

            # constants
            sel = const.tile([128, 2], F32)
            nc.vector.memset(sel[:, :], 0.0)
            nc.vector.memset(sel[0:64, 0:1], 1.0)
            nc.vector.memset(sel[64:128, 1:2], 1.0)
            gatb_bc = const.tile([128, F], F32)
            gb1 = const.tile([1, F], F32)
            nc.sync.dma_start(out=gb1[:, :], in_=gatb_d[:, :])
            nc.gpsimd.partition_broadcast(gatb_bc[:, :], gb1[:, :], channels=128)
            fcw_sb = const.tile([F, 2], F32)
            nc.sync.dma_start(out=fcw_sb[:, :], in_=fcw_d[:, :])
            fcb1 = const.tile([1, 2], F32)
            nc.sync.dma_start(out=fcb1[:, :], in_=fcb_d[:, :])
            fcb_bc = const.tile([B // N_CORES, 2], F32)
            nc.gpsimd.partition_broadcast(fcb_bc[:, :], fcb1[:, :], channels=B // N_CORES)
            adst_sb = const.tile([128, TILES * HEADS], F32)
            negbig = const.tile([128, KS], F32)
            nc.vector.memset(negbig[:, :], -BIG)

            # persistent PSUM accum for per-sample sums (feature-major)
            ps_meanT = ps_o.tile([F, 2 * TILES], F32)

            # ================= phase H: gather table =================
            # htab row = [h(128) | asrc(4) | x(32) | pad0(28)]
            NCHUNK = N // 128
            GRP = 3
            for g0 in range(0, NCHUNK, GRP):
                ks = range(g0, min(g0 + GRP, NCHUNK))
                ps = ps_h.tile([128, 136 * GRP], F32, tag="hps")
                for j, k in enumerate(ks):
                    lh = hlhs.tile([C, 128], F32)
                    nc.scalar.dma_start(out=lh[:, :], in_=xT_d[:, k * 128 : (k + 1) * 128])
                    nc.tensor.matmul(
                        ps[:, j * 136 : j * 136 + 136],
                        lhsT=lh[:, :],
                        rhs=wcombo[:, :],
                        start=True,
                        stop=True,
                    )
                hs = hstp.tile([128, ROW * GRP], F32)
                hsv = hs.rearrange("p (k w) -> p k w", w=ROW)
                nc.gpsimd.memset(hsv[:, 0 : len(ks), 132 + C + 1 : ROW], 0.0)
                for j, k in enumerate(ks):
                    nc.scalar.dma_start(
                        out=hsv[:, j, 132 : 132 + C],
                        in_=xf_d[k * 128 : (k + 1) * 128, :],
                    )
                nc.scalar.copy(
                    hsv[:, 0 : len(ks), 0:132],
                    ps.rearrange("p (k w) -> p k w", w=136)[:, 0 : len(ks), 0:132],
                )
                # fp32 |x_j|^2/2 column for the exact rescore
                nc.scalar.copy(
                    hsv[:, 0 : len(ks), 132 + C : 132 + C + 1],
                    sq2h[:, g0 : g0 + len(ks)].unsqueeze(2),
                )
                for j, k in enumerate(ks):
                    nc.sync.dma_start(
                        out=htab[k * 128 : (k + 1) * 128, :],
                        in_=hs[:, j * ROW : (j + 1) * ROW],
                    )

            # local adst slices (fp32) for the attention phase
            for t in range(TILES):
                ps2 = ps_h.tile([128, 136 * GRP], F32, tag="hps")
                nc.tensor.matmul(
                    ps2[:, 0:4],
                    lhsT=xTloc_sb[:, t * 128 : (t + 1) * 128],
                    rhs=wcombo[:, F + 4 : F + 8],
                    start=True,
                    stop=True,
                )
                nc.scalar.copy(adst_sb[:, t * 4 : t * 4 + 4], ps2[:, 0:4])

            # ================= kNN + attention per tile =================
            for t in range(TILES):
                s_sb = spool.tile([128, N], F32)
                for r in range(N // 1024):
                    ps = ps_s.tile([128, 1024], F32)
                    ach = apool.tile([C + 1, 1024], BF16)
                    dma_eng = nc.sync if r % 2 == 0 else nc.scalar
                    dma_eng.dma_start(
                        out=ach[:, :], in_=a_dram[:, r * 1024 : (r + 1) * 1024]
                    )
                    for hh in range(2):
                        nc.tensor.matmul(
                            ps[:, hh * 512 : (hh + 1) * 512],
                            lhsT=b_t[:, t * 128 : (t + 1) * 128],
                            rhs=ach[:, hh * 512 : (hh + 1) * 512],
                            start=True,
                            stop=True,
                        )
                    nc.scalar.copy(s_sb[:, r * 1024 : (r + 1) * 1024], ps[:, :])

                v8 = k8p.tile([128, 8], F32)
                nc.vector.max(out=v8[:, :], in_=s_sb[:, :])
                i8 = k8p.tile([128, 8], U16)
                nc.vector.max_index(out=i8[:, :], in_max=v8[:, :], in_values=s_sb[:, :])

                # wrap indices for dma_gather: L[i= j*128+p] at (i%16, i//16),
                # replicated into all 8 16-partition groups, via a DRAM bounce.
                iscr = dscr.tile([128 * KS], I16)
                nc.gpsimd.dma_start(
                    out=iscr.rearrange("(j p) -> p j", p=128),
                    in_=i8[:, :].bitcast(I16),
                )
                lrep = idxp.tile([128, 8 * KS], I16)
                for rep in range(8):
                    nc.gpsimd.dma_start(
                        out=lrep[16 * rep : 16 * (rep + 1), :],
                        in_=iscr.rearrange("(s q) -> q s", q=16),
                    )
                # gather 1: x rows of the 8 candidates (exact rescore)
                g_a = gpool.tile([128, KS * 64], F32, tag="ga")
                nc.gpsimd.dma_gather(
                    out_ap=g_a.rearrange("p (j e) -> p j e", j=KS),
                    in_ap=htab[:, F:ROW],
                    idxs_ap=lrep[:, :],
                    num_idxs=128 * KS,
                    num_idxs_reg=128 * KS,
                    elem_size=64,
                    elem_step=ROW,
                )
                # gather 2: full rows (h | asrc) for attention
                g_t = gpool.tile([128, KS * ROW], F32, tag="gt")
                nc.gpsimd.dma_gather(
                    out_ap=g_t.rearrange("p (j e) -> p j e", j=KS),
                    in_ap=htab[:, :],
                    idxs_ap=lrep[:, :],
                    num_idxs=128 * KS,
                    num_idxs_reg=128 * KS,
                    elem_size=ROW,
                )

                # ---- exact rescore of the 8 candidates ----
                gav = g_a.rearrange("p (j e) -> p j e", j=KS)  # [128, 8, 64]
                xrow = k8p.tile([128, C], F32)
                nc.sync.dma_start(
                    out=xrow[:, :], in_=xfloc_d[t * 128 : (t + 1) * 128, :]
                )
                half_t = attp.tile([128, KS * C], F32, tag="big")
                # s8 = sum_c x_i*x_j - |x_j|^2/2  (sequential fp32, matches the
                # PE's accumulation on near-ties)
                nc.vector.tensor_mul(
                    half_t.rearrange("p (j c) -> p j c", j=KS),
                    gav[:, :, 4 : 4 + C],
                    xrow.unsqueeze(1).to_broadcast([128, KS, C]),
                )
                s8 = attp.tile([128, KS], F32)
                nc.vector.tensor_reduce(
                    s8[:, :],
                    half_t.rearrange("p (j c) -> p j c", j=KS),
                    axis=AX.X,
                    op=ALU.add,
                )
                nc.vector.tensor_sub(
                    s8[:, :], s8[:, :], gav[:, :, 4 + C : 5 + C].squeeze(2)
                )
                # drop duplicate candidate slots (equal coarse values collapse
                # to the same index in find_index8); slots sorted by value, so
                # duplicates are adjacent.
                nodup = attp.tile([128, KS], mybir.dt.uint8)
                nc.vector.memset(nodup[:, 0:1], 1)
                nc.vector.tensor_tensor(
                    nodup[:, 1:KS], i8[:, 1:KS], i8[:, 0 : KS - 1], op=ALU.not_equal
                )
                # s8m = nodup ? s8 : -BIG (exact copy; no precision loss), then
                # tau = 5th largest ; m01 = s8m >= tau
                s8m = attp.tile([128, KS], F32)
                nc.vector.select(s8m[:, :], nodup[:, :], s8[:, :], negbig[:, :])
                v8f = attp.tile([128, 8], F32)
                nc.vector.max(out=v8f[:, :], in_=s8m[:, :])
                m01 = attp.tile([128, KS], F32)
                nc.vector.tensor_scalar(
                    m01[:, :], s8m[:, :], v8f[:, 4:5], None, op0=ALU.is_ge
                )

                if debug:
                    nc.sync.dma_start(
                        out=dbg_i8[t * 128 : (t + 1) * 128, :], in_=i8[:, :]
                    )
                    nc.sync.dma_start(
                        out=dbg_s8[t * 128 : (t + 1) * 128, :], in_=s8[:, :]
                    )
                    nc.sync.dma_start(
                        out=dbg_m01[t * 128 : (t + 1) * 128, :], in_=m01[:, :]
                    )

                # ---- attention over the 8 slots (masked softmax) ----
                gv = g_t.rearrange("p (j e) -> p j e", j=KS)
                e_t = attp.tile([128, KS * HEADS], F32)
                nc.vector.tensor_add(
                    e_t.rearrange("p (j h) -> p j h", j=KS),
                    gv[:, :, F : F + 4],
                    adst_sb[:, t * 4 : t * 4 + 4]
                    .unsqueeze(1)
                    .to_broadcast([128, KS, HEADS]),
                )
                # leaky relu = max(0.2*e, e), in place
                nc.vector.scalar_tensor_tensor(
                    e_t[:, :], e_t[:, :], NEG_SLOPE, e_t[:, :], op0=ALU.mult, op1=ALU.max
                )
                nc.scalar.activation(e_t[:, :], e_t[:, :], ACT.Exp)
                pm = attp.tile([128, KS * HEADS], F32)
                nc.vector.tensor_mul(
                    pm.rearrange("p (j h) -> p j h", j=KS),
                    e_t.rearrange("p (j h) -> p j h", j=KS),
                    m01.unsqueeze(2).to_broadcast([128, KS, HEADS]),
                )
                zz = attp.tile([128, HEADS], F32)
                nc.vector.tensor_reduce(
                    zz[:, :],
                    pm.rearrange("p (j h) -> p h j", j=KS),
                    axis=AX.X,
                    op=ALU.add,
                )
                zr = attp.tile([128, HEADS], F32)
                nc.vector.reciprocal(zr[:, :], zz[:, :])
                w1 = attp.tile([128, KS * F], F32, tag="big")
                nc.vector.tensor_mul(
                    w1.rearrange("p (j h d) -> p j h d", j=KS, h=HEADS),
                    g_t.rearrange("p (j hh d) -> p j hh d", j=KS, d=HID)[:, :, 0:HEADS, :],
                    pm.rearrange("p (j h) -> p j h", j=KS).to_broadcast(
                        [128, KS, HEADS, HID]
                    ),
                )
                att_t = attp.tile([128, F], F32)
                nc.vector.tensor_reduce(
                    att_t.rearrange("p (h d) -> p h d", h=HEADS),
                    w1.rearrange("p (j h d) -> p h d j", j=KS, h=HEADS),
                    axis=AX.X,
                    op=ALU.add,
                )
                nc.vector.tensor_mul(
                    att_t.rearrange("p (h d) -> p h d", h=HEADS),
                    att_t.rearrange("p (h d) -> p h d", h=HEADS),
                    zr.to_broadcast([128, HEADS, HID]),
                )
                nc.vector.tensor_add(att_t[:, :], att_t[:, :], gatb_bc[:, :])
                # elu(z) = max(z,0) + exp(min(z,0)) - 1
                zn = attp.tile([128, F], F32)
                nc.vector.tensor_scalar_min(zn[:, :], att_t[:, :], 0.0)
                nc.scalar.activation(zn[:, :], zn[:, :], ACT.Exp)
                nc.vector.tensor_scalar_max(att_t[:, :], att_t[:, :], 0.0)
                zelu = attp.tile([128, F], F32)
                nc.vector.scalar_tensor_tensor(
                    zelu[:, :], zn[:, :], -1.0, att_t[:, :], op0=ALU.add, op1=ALU.add
                )
                # per-sample sums (each tile = 2 samples), feature-major
                nc.tensor.matmul(
                    ps_meanT[:, 2 * t : 2 * t + 2],
                    lhsT=zelu[:, :],
                    rhs=sel[:, :],
                    start=True,
                    stop=True,
                )

            # ================= output =================
            meanT = const.tile([F, 2 * TILES], F32)
            nc.scalar.copy(meanT[:, :], ps_meanT[:, :])
            ps_fc = ps_o.tile([B // N_CORES, 2], F32)
            nc.tensor.matmul(
                ps_fc[:, :], lhsT=meanT[:, :], rhs=fcw_sb[:, :], start=True, stop=True
            )
            y_pre = const.tile([B // N_CORES, 2], F32)
            nc.scalar.mul(y_pre[:, :], ps_fc[:, :], 1.0 / E)
            y_sb = const.tile([B // N_CORES, 2], F32)
            nc.vector.tensor_add(y_sb[:, :], y_pre[:, :], fcb_bc[:, :])
            nc.sync.dma_start(out=y_d[:, :], in_=y_sb[:, :])

    nc.finalize()
    return nc


_NC = None


def _get_nc():
    global _NC
    if _NC is None:
        _NC = build_bass()
    return _NC


def _prep_inputs(inputs):
    x = np.ascontiguousarray(inputs["x"], dtype=np.float32)
    xf = np.ascontiguousarray(x.reshape(N, C))
    xT = np.ascontiguousarray(xf.T)
    shared = {
        "xf": xf,
        "xT": xT,
        "gat_w": np.ascontiguousarray(inputs["gat_w"], np.float32),
        "gat_asrc": np.ascontiguousarray(
            np.asarray(inputs["gat_asrc"], np.float32).reshape(1, F)
        ),
        "gat_adst": np.ascontiguousarray(
            np.asarray(inputs["gat_adst"], np.float32).reshape(1, F)
        ),
        "gat_b": np.ascontiguousarray(
            np.asarray(inputs["gat_b"], np.float32).reshape(1, F)
        ),
        "fc_w": np.ascontiguousarray(inputs["fc_w"], np.float32),
        "fc_b": np.ascontiguousarray(
            np.asarray(inputs["fc_b"], np.float32).reshape(1, 2)
        ),
    }
    maps = []
    for c in range(N_CORES):
        m = dict(shared)
        m["xTloc"] = np.ascontiguousarray(xT[:, c * LOCAL : (c + 1) * LOCAL])
        m["xfloc"] = np.ascontiguousarray(xf[c * LOCAL : (c + 1) * LOCAL, :])
        maps.append(m)
    return maps


def kernel(**inputs) -> np.ndarray:
    nc = _get_nc()
    maps = _prep_inputs(inputs)
    res = run_bass_kernel_spmd(
        nc,
        maps,
        core_ids=list(range(N_CORES)),
        trace=bool(int(os.environ.get("KERNEL_TRACE", "0"))),
    )
    y = np.concatenate([res.results[c]["y"] for c in range(N_CORES)], axis=0)
    kernel.last_exec_time_ns = res.exec_time_ns
    kernel.last_results = res
    return y
